# revision 1
# baseline (speedup 1.0000x reference)
"""Trainium2 Bass kernel for nn_BinarizedLinearBlock.

Computes y = clip(BatchNorm1d(x) @ sign(W)^T, -1, 1) for
x [8192, 2048] f32, W [2048, 2048] f32, gamma/beta [2048] f32.

Strategy (8 NeuronCores, data-parallel over batch):
  - Each core gets a batch shard x_j [1024, 2048] and the full weight.
  - BN statistics need the whole batch: each core computes partial
    (sum, sumsq) per feature from its shard, a 16 KB AllReduce combines
    them, then each core normalizes its shard locally.
  - Both matmul operands need the contraction dim (IN) on partitions,
    so x and W are transposed on the PE (128x128 identity matmuls,
    4 per PSUM bank).  16-bit transposes: fp32 inputs are pre-cast on
    DVE (GpSimd casts are 7 us/tile, DVE ~1 us; fp32 PE transposes pay
    2 cycles/row).
  - Loads are plain fp32 on both HWDGE rings (SWDGE dtype-cast DMAs
    collapse to ~115 GB/s single-queue; W column-slices degrade to
    512 B packets, so W loads natural row-tiles).  x is queued ahead
    of W in both ring FIFOs; y stores follow on the sync ring.
  - W path: DVE cast fp32->bf16 (keeps fp32 exponent range so sign()
    is never corrupted by underflow), PE transpose, ACT Sign on the
    PSUM eviction emits exact +-1 in fp16.
  - The AllReduce bounce rides the otherwise-empty SWDGE queue.  The
    single ACT Sqrt is emitted between the two W sign batches so it
    never blocks the ACT queue.
  - Main matmul: lhsT = xn^T tile (fp16), rhs = sign(W)^T (fp16),
    fp32 PSUM accumulation over 16 k-tiles; eviction fuses the
    hardtanh clip via one DVE tensor_scalar (min 1, max -1).
  - h-outer matmul loop: the first output half consumes only W
    o-tiles 0-7, so matmuls start while o-tiles 8-15 still stream.
"""

import sys

sys.path.insert(0, "/opt/trn_rl_repo")

import numpy as np
import ml_dtypes

import concourse.bass as bass
import concourse.bacc as bacc
import concourse.mybir as mybir
import concourse.tile as tile
from concourse.bass_utils import run_bass_kernel_spmd

F32 = mybir.dt.float32
F16 = mybir.dt.float16
BF16 = mybir.dt.bfloat16
ALU = mybir.AluOpType
AFT = mybir.ActivationFunctionType

B, IN, OUT = 8192, 2048, 2048
NCORES = 8
BSH = B // NCORES          # 1024 batch rows per core
KB = BSH // 128            # 8 batch tiles per core
KI = IN // 128             # 16 contraction (input-feature) tiles
KO = OUT // 128            # 16 output-feature (W row) tiles
BN_EPS = 1e-5


def build_kernel_body(tc, y_d, x_d, w_d, gam_d, bet_d, idf_d, idb_d):
    nc = tc.nc

    consts = tc.tile_pool(name="consts", bufs=1)
    persist = tc.tile_pool(name="persist", bufs=1)
    xnat_pool = tc.tile_pool(name="xnat", bufs=3)
    xnat16_pool = tc.tile_pool(name="xnat16", bufs=3)
    wstg_pool = tc.tile_pool(name="wstg", bufs=3)
    wstg16_pool = tc.tile_pool(name="wstg16", bufs=3)
    ysb_pool = tc.tile_pool(name="ysb", bufs=3)
    tpsum = tc.tile_pool(name="tpsum", bufs=2, space="PSUM")
    ypsum = tc.tile_pool(name="ypsum", bufs=2, space="PSUM")
    dram = tc.tile_pool(name="dram", bufs=1, space="DRAM")

    ctxs = [consts, persist, xnat_pool, xnat16_pool, wstg_pool, wstg16_pool,
            ysb_pool, tpsum, ypsum, dram]
    entered = [c.__enter__() for c in ctxs]
    (consts, persist, xnat_pool, xnat16_pool, wstg_pool, wstg16_pool,
     ysb_pool, tpsum, ypsum, dram) = entered

    # ---- constants -------------------------------------------------
    ident_f = consts.tile([128, 128], F16)
    ident_b = consts.tile([128, 128], BF16)
    gamma_sb = consts.tile([128, KI], F32)
    beta_sb = consts.tile([128, KI], F32)
    zero_col = consts.tile([128, 1], F32)
    eps_col = consts.tile([128, 1], F32)
    nc.vector.memset(zero_col[:], 0.0)
    nc.vector.memset(eps_col[:], BN_EPS)
    nc.scalar.dma_start(ident_f[:], idf_d[:, :])
    nc.scalar.dma_start(ident_b[:], idb_d[:, :])
    nc.scalar.dma_start(gamma_sb[:], gam_d[:, :])
    nc.scalar.dma_start(beta_sb[:], bet_d[:, :])

    # ---- persistent SBUF tensors ----------------------------------
    xT3 = persist.tile([128, KI, BSH], F16)       # x^T, later xn^T in place
    wbT3 = persist.tile([128, KI, OUT], F16)      # sign(W)^T, exact +-1

    # ---- Phase X: load x fp32 (both rings), DVE cast fp16, --------
    # PE transpose (4 per PSUM bank), DVE evict, bn_stats in four
    # quarter-sweeps so only ~5 us of stats sit after the last x tile
    bnst = persist.tile([128, KI, 4, 6], F32)
    bnag = persist.tile([128, KI, 2], F32)
    for b in range(KB):
        xnat = xnat_pool.tile([128, IN], F32)
        eng = nc.sync if b % 2 == 0 else nc.scalar
        eng.dma_start(xnat[:], x_d[b * 128:(b + 1) * 128, :])
        xnat16 = xnat16_pool.tile([128, IN], F16)
        nc.vector.tensor_copy(xnat16[:], xnat[:])
        for tg in range(KI // 4):
            t = tg * 4
            ps = tpsum.tile([128, 4, 128], F16, tag="xT")
            for j in range(4):
                nc.tensor.transpose(
                    ps[:, j, :], xnat16[:, (t + j) * 128:(t + j + 1) * 128],
                    ident_f[:]
                )
            nc.vector.tensor_copy(xT3[:, t:t + 4, b * 128:(b + 1) * 128], ps[:])
        if b % 2 == 1:
            ch = b // 2
            for t in range(KI):
                nc.vector.bn_stats(
                    bnst[:, t, ch, :], xT3[:, t, ch * 256:(ch + 1) * 256]
                )
    for t in range(KI):
        nc.vector.bn_aggr(bnag[:, t, :], bnst[:, t, :, :])

    # local sums for the AllReduce: s1 = mean * BSH ; s2 = (var + mean^2) * BSH
    stats = persist.tile([128, 2 * KI], F32)
    means = bnag[:, :, 0]
    vars_ = bnag[:, :, 1]
    msq = persist.tile([128, KI], F32)
    nc.vector.tensor_scalar(stats[:, 0:KI], means, float(BSH), None, op0=ALU.mult)
    nc.vector.tensor_tensor(msq[:], means, means, op=ALU.mult)
    nc.vector.tensor_tensor(msq[:], vars_, msq[:], op=ALU.add)
    nc.vector.tensor_scalar(stats[:, KI:2 * KI], msq[:], float(BSH), None, op0=ALU.mult)

    # ---- Phase R: AllGather over the 8 cores (SWDGE bounce) -------
    # AllGather has a ~2x lower latency floor than AllReduce at this
    # size (16 KB); the 8-way reduction happens locally on DVE.
    cc_in = dram.tile([128, 2 * KI], F32)
    # rank j's gathered copy sits at byte offset j*16KB = rows
    # [j*128:(j+1)*128] of a [NCORES*128, 2*KI] row-major buffer
    cc_out = dram.tile([NCORES * 128, 2 * KI], F32)
    nc.gpsimd.dma_start(cc_in[:], stats[:])
    nc.gpsimd.collective_compute(
        "AllGather",
        ALU.bypass,
        replica_groups=[list(range(NCORES))],
        ins=[cc_in[:].opt()],
        outs=[cc_out[:].opt()],
    )

    # ---- Phase W: natural fp32 row-tiles on both rings (queued ----
    # after x), DVE cast to bf16, PE transpose, ACT Sign eviction.
    def w_prep(o):
        wstg = wstg_pool.tile([128, IN], F32, name=f"wstg{o}", tag="wstg")
        eng = nc.sync if o % 2 == 0 else nc.scalar
        eng.dma_start(wstg[:], w_d[o * 128:(o + 1) * 128, :])
        wstg16 = wstg16_pool.tile([128, IN], BF16, name=f"wstg16_{o}", tag="wstg16")
        nc.vector.tensor_copy(wstg16[:], wstg[:])
        for tg in range(KI // 4):
            t = tg * 4
            ps = tpsum.tile([128, 4, 128], BF16, tag="wT", name=f"psw{o}_{tg}")
            for j in range(4):
                nc.tensor.transpose(
                    ps[:, j, :], wstg16[:, (t + j) * 128:(t + j + 1) * 128],
                    ident_b[:]
                )
            nc.scalar.sign(
                wbT3[:, t:t + 4, o * 128:(o + 1) * 128],
                ps[:],
                bias=zero_col[:],
            )

    for o in range(KO // 2):
        w_prep(o)

    ag_sb = persist.tile([128, NCORES, 2 * KI], F32)
    nc.gpsimd.dma_start(
        ag_sb[:], cc_out[:].rearrange("(j p) c -> p j c", p=128)
    )
    gstats = persist.tile([128, 2 * KI], F32)
    nc.vector.tensor_tensor(gstats[:], ag_sb[:, 0, :], ag_sb[:, 1, :], op=ALU.add)
    for j in range(2, NCORES):
        nc.vector.tensor_tensor(gstats[:], gstats[:], ag_sb[:, j, :], op=ALU.add)

    # ---- Phase N: a = gamma * rsqrt(var+eps); c = beta - mean * a -
    # (the lone ACT Sqrt sits between the two sign batches, so it
    # waits for the AllReduce without blocking any sign eviction)
    meang = persist.tile([128, KI], F32)
    ex2g = persist.tile([128, KI], F32)
    varg = persist.tile([128, KI], F32)
    stdg = persist.tile([128, KI], F32)
    invg = persist.tile([128, KI], F32)
    a_sc = persist.tile([128, KI], F32)
    c_sc = persist.tile([128, KI], F32)
    nc.vector.tensor_scalar(meang[:], gstats[:, 0:KI], 1.0 / B, None, op0=ALU.mult)
    nc.vector.tensor_scalar(ex2g[:], gstats[:, KI:2 * KI], 1.0 / B, None, op0=ALU.mult)
    nc.vector.tensor_tensor(varg[:], meang[:], meang[:], op=ALU.mult)
    nc.vector.tensor_tensor(varg[:], ex2g[:], varg[:], op=ALU.subtract)
    nc.scalar.activation(stdg[:], varg[:], AFT.Sqrt, bias=eps_col[:])
    nc.vector.reciprocal(invg[:], stdg[:])
    nc.vector.tensor_tensor(a_sc[:], gamma_sb[:], invg[:], op=ALU.mult)
    nc.vector.tensor_tensor(c_sc[:], meang[:], a_sc[:], op=ALU.mult)
    nc.vector.tensor_tensor(c_sc[:], beta_sb[:], c_sc[:], op=ALU.subtract)

    for t in range(KI):
        nc.vector.tensor_scalar(
            xT3[:, t, :], xT3[:, t, :],
            a_sc[:, t:t + 1], c_sc[:, t:t + 1],
            op0=ALU.mult, op1=ALU.add,
        )

    for o in range(KO // 2, KO):
        w_prep(o)

    # ---- Phase M: main matmul + fused clip eviction ---------------
    for h in range(2):
        for b in range(KB):
            yp = ypsum.tile([128, 1024], F32)
            for t in range(KI):
                lhs = xT3[:, t, b * 128:(b + 1) * 128]
                for n2 in range(2):
                    nc.tensor.matmul(
                        yp[:, n2 * 512:(n2 + 1) * 512],
                        lhs,
                        wbT3[:, t, h * 1024 + n2 * 512: h * 1024 + (n2 + 1) * 512],
                        start=(t == 0),
                        stop=(t == KI - 1),
                    )
            ysb = ysb_pool.tile([128, 1024], F32)
            nc.vector.tensor_scalar(
                ysb[:], yp[:], 1.0, -1.0, op0=ALU.min, op1=ALU.max
            )
            nc.sync.dma_start(
                y_d[b * 128:(b + 1) * 128, h * 1024:(h + 1) * 1024], ysb[:]
            )

    for c in reversed(ctxs):
        c.__exit__(None, None, None)


def build_program():
    nc = bacc.Bacc(
        "TRN2",
        target_bir_lowering=False,
        debug=False,
        num_devices=NCORES,
    )
    x_d = nc.dram_tensor("x", [BSH, IN], F32, kind="ExternalInput")
    w_d = nc.dram_tensor("weight", [OUT, IN], F32, kind="ExternalInput")
    gam_d = nc.dram_tensor("gamma_blk", [128, KI], F32, kind="ExternalInput")
    bet_d = nc.dram_tensor("beta_blk", [128, KI], F32, kind="ExternalInput")
    idf_d = nc.dram_tensor("ident_f16", [128, 128], F16, kind="ExternalInput")
    idb_d = nc.dram_tensor("ident_bf16", [128, 128], BF16, kind="ExternalInput")
    y_d = nc.dram_tensor("y", [BSH, OUT], F32, kind="ExternalOutput")

    with tile.TileContext(nc) as tc:
        build_kernel_body(
            tc, y_d[:, :], x_d[:, :], w_d[:, :], gam_d[:, :], bet_d[:, :],
            idf_d[:, :], idb_d[:, :],
        )
    nc.compile()
    return nc


_CACHE = {}


def _get_program():
    if "nc" not in _CACHE:
        _CACHE["nc"] = build_program()
    return _CACHE["nc"]


def make_in_maps(x, weight, gamma, beta):
    x = np.ascontiguousarray(np.asarray(x, dtype=np.float32))
    weight = np.ascontiguousarray(np.asarray(weight, dtype=np.float32))
    gamma = np.asarray(gamma, dtype=np.float32)
    beta = np.asarray(beta, dtype=np.float32)
    gamma_blk = np.ascontiguousarray(gamma.reshape(KI, 128).T)
    beta_blk = np.ascontiguousarray(beta.reshape(KI, 128).T)
    ident_f = np.eye(128, dtype=np.float16)
    ident_b = np.eye(128, dtype=ml_dtypes.bfloat16)
    in_maps = []
    for j in range(NCORES):
        in_maps.append({
            "x": np.ascontiguousarray(x[j * BSH:(j + 1) * BSH]),
            "weight": weight,
            "gamma_blk": gamma_blk,
            "beta_blk": beta_blk,
            "ident_f16": ident_f,
            "ident_bf16": ident_b,
        })
    return in_maps


def run(x, weight, gamma, beta, **spmd_kwargs):
    """Run on hardware; returns (y_full, BassKernelResults)."""
    nc = _get_program()
    in_maps = make_in_maps(x, weight, gamma, beta)
    res = run_bass_kernel_spmd(nc, in_maps, core_ids=list(range(NCORES)), **spmd_kwargs)
    y = np.concatenate([r["y"] for r in res.results], axis=0)
    return np.asarray(y, dtype=np.float32), res


def run_traced(x, weight, gamma, beta, profile_dir=None):
    """Run with NTFF capture via the axon sidechannel; returns
    (y_full, per_core_exec_ns, profile_dir)."""
    import ctypes, tempfile
    from concourse import bass2jax
    import gauge.profiler
    from concourse._compat import FishPath

    nc = _get_program()
    in_maps = make_in_maps(x, weight, gamma, beta)

    lib = ctypes.CDLL("/opt/axon/libaxon_pjrt.so")
    lib.axon_start_nrt_profile.argtypes = [
        ctypes.POINTER(ctypes.c_int64), ctypes.c_size_t]
    lib.axon_start_nrt_profile.restype = ctypes.c_int64
    lib.axon_stop_nrt_profile.argtypes = [ctypes.c_char_p]
    lib.axon_stop_nrt_profile.restype = ctypes.c_int64

    if profile_dir is None:
        profile_dir = tempfile.mkdtemp(prefix="ntff_")
    rc = lib.axon_start_nrt_profile(None, 0)
    assert rc == 0, f"axon_start_nrt_profile rc={rc}"
    try:
        results = bass2jax.run_bass_via_pjrt(nc, in_maps, n_cores=NCORES)
    finally:
        n = lib.axon_stop_nrt_profile(profile_dir.encode())
    y = np.concatenate([r["y"] for r in results], axis=0)
    if n <= 0:
        return np.asarray(y, dtype=np.float32), None, profile_dir

    profile = gauge.profiler.Profile(
        profile_path=FishPath(profile_dir),
        kernel_dev_mode=True,
        profile_on_exit=False,
        bass_kernel=nc.m,
        offline_processing=True,
        fname="*_body*",
    )
    perfetto_results = profile.to_perfetto(model_index=tuple(range(NCORES)))
    exec_ns = {}
    for i, pr in enumerate(perfetto_results or []):
        exec_ns[i] = pr.exec_time_ns
    return np.asarray(y, dtype=np.float32), exec_ns, profile_dir


def kernel(x, weight, gamma, beta):
    y, _ = run(x, weight, gamma, beta)
    return y

